# revision 11
# baseline (speedup 1.0000x reference)
"""Multi-head attention (QKV projection + softmax attention) on 8 TRN2 NeuronCores.

Problem: x[2,2048,1024] @ W_qkv[1024,3072] + b -> q,k,v (16 heads, d=64);
out = softmax(q k^T / sqrt(d)) v, returned as [2,2048,1024] fp32.

Sharding: head-parallel. Core c owns heads {2c, 2c+1} for both batches.
Each core computes the QKV projection only for its own heads' columns of
W_qkv and runs its 4 (batch, head) attention instances locally. No
collectives; host gathers/normalizes/concatenates.

Device-side layout (per core):
  - x is fed pre-transposed AND pre-cast to fp16 (xT [B, C, N]) so the
    projection produces qT/kT [d, n] (head dim on partitions, h0 on 0-63,
    h1 on 64-127) directly with full-width matmuls.
  - All matmul operands are fp16 (PSUM accumulation stays fp32). fp16 runs
    at the same 1 col/cycle PE rate as f32r but gets FWL (fast weight
    load, 2 elems per 32-bit read) and halves SBUF/DMA traffic. Numerics
    sim: end-to-end max rel err ~1.2e-3 vs the fp32 reference (gate 2e-2).
  - The two heads' QK matmuls are row-tiled (h0 rows 0-63, h1 rows 64-127,
    via base_partition-inferred tile_position), so they run concurrently
    on disjoint PE row strips.
  - Attention computes S^T = K Q^T per head via matmul(lhsT=kT, rhs=qT),
    softmax numerator via exp per j-chunk covering both heads, and
    out^T = V^T E^T via matmul(lhsT=[v|1], rhs=E^T) accumulated over j in
    PSUM. The appended ones-column yields the softmax denominator as row
    64 of the accumulator; normalization happens on host.
  - The v bias is NOT applied on device: softmax weights sum to 1, so
    out = sum_k w_k (v_k + bv) = (sum_k w_k v_k) + bv, and the host adds
    bv after normalization.

Performance structure (found via on-HW A/B on the f32r baseline): engine
throughputs are fine, but the per-j QK->exp->PV cross-engine handoff
latency is very expensive on this hardware. Two mitigations:
  - PV consumes e with a lag of 3 j-chunks (deep software pipeline).
  - every DVE_EVERY-th j-chunk's exp runs on the Vector engine instead of
    ACT, via two custom DVE ops computing (L(s)*(K(s)*c)^4)^8 = K^32*L^8
    ~ exp(s) to 1e-5 rel (exactly scale-1 so ACT/DVE rows mix freely in
    one softmax column). This splits the s-buffer WAR chain across two
    independent engines, amortizing the handoff latency.
"""

import math
import os
from contextlib import ExitStack
from dataclasses import dataclass

import numpy as np

import concourse.bass as bass
import concourse.tile as tile
from concourse import bacc, mybir
from concourse.bass_utils import run_bass_kernel_spmd
from concourse.masks import make_identity

F32 = mybir.dt.float32
F16 = mybir.dt.float16

# ---------------------------------------------------------------------------
# Custom DVE exp: e = (L(s) * (K(s)*c)^4)^8 = K^32 * L^8 ~ exp(s), |s|<=8.5
#   K(s) = ((s+A)^2 + B) * (s+D)   op1 EXPK4_ANT: ((K*c))^4   (8 ALU ops)
#   L(s) = ((s+E)^2 + F) * G       op2 EXPFIN_ANT: (L*Src1)^8 (8 ALU ops)
# c = 2^-18 keeps intermediates O(1); G folds the fit constant so the
# overall scale is exactly 1. fp32 end-to-end max rel err vs exp: 9.6e-6.

EXP_A = 40.82381078942194
EXP_B = 2861.2185768771237
EXP_D = 63.39710432711229
EXP_C = 3.814697265625e-06      # 2^-18
EXP_E = -16.140267142823447
EXP_F = 2896.6915957224187
EXP_G = 0.00022030800137427044

_DVE_OPS = {}


def _register_dve_exp_ops():
    if _DVE_OPS:
        return
    import concourse.dve_ops as dve_ops_mod
    from concourse.dve_ops import OPS, DveOp
    from concourse.dve_spec import (Spec, Src0, Src1, C0, C1, C2, C3, sq,
                                    lower, _spill_c3_to_src1)
    from concourse.dve_uop import DveOpSpec

    by_name = {op.name: op for op in OPS}
    if "EXPK4_ANT" in by_name:
        _DVE_OPS["k4"] = by_name["EXPK4_ANT"]
        _DVE_OPS["fin"] = by_name["EXPFIN_ANT"]
        return

    k4_body = _spill_c3_to_src1(
        sq(sq((sq(Src0 + C0) + C1) * ((Src0 + C2) * C3))))
    K4 = DveOp(
        "EXPK4_ANT",
        Spec(
            body=k4_body,
            reference=lambda in0, in1, s0, s1, imm2: (
                (((in0 + s0) ** 2 + s1) * ((in0 + imm2) * in1)) ** 4),
        ),
        subdim=False,
        uops_sha={},
    )
    fin_body = sq(sq(sq(((sq(Src0 + C0) + C1) * C2) * Src1)))
    FIN = DveOp(
        "EXPFIN_ANT",
        Spec(
            body=fin_body,
            reference=lambda in0, in1, s0, s1, imm2: (
                ((((in0 + s0) ** 2 + s1) * imm2) * in1) ** 8),
        ),
        subdim=False,
        uops_sha={},
    )
    for op in (K4, FIN):
        for ver in ("v3", "v4"):
            spec = DveOpSpec(name=op.name, opcode=1,
                             uops=lower(op.spec, ver=ver), rd1_en=True)
            op.uops_sha[ver] = spec.sha(ver)
    OPS.append(K4)
    OPS.append(FIN)
    base = dve_ops_mod._CUSTOM_DVE_ROW_BASE
    for i, op in enumerate(OPS):
        dve_ops_mod._SUB_OPCODE_FOR_NAME[op.name] = base + i
    assert max(dve_ops_mod._SUB_OPCODE_FOR_NAME.values()) < 0x20
    _DVE_OPS["k4"] = K4
    _DVE_OPS["fin"] = FIN


def _emit_dve_exp(nc, k4_tile, out, s_ap, c_tile):
    """out (fp16 SBUF) = exp(s_ap); k4_tile fp32 scratch, c_tile [P,1]=c."""
    nc.vector._custom_dve(_DVE_OPS["k4"], out=k4_tile, in0=s_ap, in1=c_tile,
                          s0=EXP_A, s1=EXP_B, imm2=EXP_D)
    nc.vector._custom_dve(_DVE_OPS["fin"], out=out, in0=s_ap, in1=k4_tile,
                          s0=EXP_E, s1=EXP_F, imm2=EXP_G)


@dataclass(frozen=True)
class Cfg:
    B: int = 2          # batches
    N: int = 2048       # sequence length
    C: int = 1024       # model dim (contraction dim of the projection)
    D: int = 64         # head dim
    IT: int = 512       # i-tile (query block, moving-dim of S^T / PV matmuls)
    P: int = 128        # partitions
    LAG: int = 3        # PV consumes e this many j-chunks behind exp
    EBUFS: int = 5      # e-tile ring depth
    DVE_EVERY: int = 4  # every k-th j-chunk's exp runs on the DVE
    EXP_STUB: int = 0   # bench-only: exp writes just 1 column (timing A/B)
    QK_SPLIT: int = 1   # bench: split each QK matmul into this many pieces
    PV_SPLIT: int = 1   # bench: split each PV matmul into this many pieces
    DMA_T: int = 0      # transpose v via DMA XBAR instead of the PE
    SPLIT_POOLS: int = 1  # PV accumulators get their own PSUM ring (2 banks)

    @property
    def KO(self):       # k-chunks in the projection contraction
        return self.C // self.P

    @property
    def NJ(self):       # key chunks of 128
        return self.N // self.P

    @property
    def NI(self):       # query tiles
        return self.N // self.IT

    @property
    def HD2(self):      # two heads stacked on partitions
        return 2 * self.D


def build_attention(tc: tile.TileContext, io: dict, cfg: Cfg, repeat: int = 1):
    nc = tc.nc
    P, D, IT = cfg.P, cfg.D, cfg.IT
    xT, wq, wk, wv, bq, bk, outT = (
        io["xT"], io["wq"], io["wk"], io["wv"], io["bq"], io["bk"], io["outT"],
    )

    with ExitStack() as ctx:
        consts = ctx.enter_context(tc.tile_pool(name="consts", bufs=1))
        xpool = ctx.enter_context(tc.tile_pool(name="xpool", bufs=1))
        qkv = ctx.enter_context(tc.tile_pool(name="qkv", bufs=2))
        epool = ctx.enter_context(tc.tile_pool(name="epool", bufs=cfg.EBUFS))
        opool = ctx.enter_context(tc.tile_pool(name="opool", bufs=4))
        k4pool = ctx.enter_context(tc.tile_pool(name="k4pool", bufs=2))
        # PSUM budget (8 banks): spool 2x [128, 2*IT] = 4 banks; the PV
        # accumulators get their own 2-buf ring (they are held for a whole
        # i-row, so sharing a ring with short-lived proj/transpose tiles
        # would inject long WAR stalls into the in-order PE queue); the
        # projection psum groups and transpose outputs share a 2-buf ring.
        if cfg.SPLIT_POOLS:
            ppool = ctx.enter_context(tc.tile_pool(name="ppool", bufs=2, space="PSUM"))
            spool = ctx.enter_context(tc.tile_pool(name="spool", bufs=2, space="PSUM"))
            apool = ctx.enter_context(tc.tile_pool(name="apool", bufs=2, space="PSUM"))
        else:
            ppool = ctx.enter_context(tc.tile_pool(name="ppool", bufs=4, space="PSUM"))
            spool = ctx.enter_context(tc.tile_pool(name="spool", bufs=2, space="PSUM"))
            apool = ppool

        identity = consts.tile([P, P], F16)
        make_identity(nc, identity)

        c_tile = consts.tile([P, 1], F32, name="c_exp")
        nc.vector.memset(c_tile, EXP_C)

        # weights [ki, ko, m] fp16 (host pre-cast) and biases [p, 1] fp32.
        w_sb = {}
        for name, wdram in (("q", wq), ("k", wk), ("v", wv)):
            w_sb[name] = consts.tile([P, cfg.KO, cfg.HD2], F16, name=f"w_{name}")
            nc.sync.dma_start(
                out=w_sb[name],
                in_=wdram.rearrange("(ko ki) m -> ki ko m", ki=P),
            )
        b_sb = {}
        for name, bdram in (("q", bq), ("k", bk)):
            b_sb[name] = consts.tile([cfg.HD2, 1], F32, name=f"b_{name}")
            nc.sync.dma_start(out=b_sb[name], in_=bdram)

        DA = D + 1  # head-dim columns + appended ones column

        from collections import deque

        work_q = deque()

        def start_job(rep, b, first=False):
            """Allocate batch tiles, emit the x-load DMAs, and enqueue the
            projection/transpose work thunks. The matmuls drain into the
            surrounding attention loops; emission order is program order, so
            attention uses ensure() to pull required producers through the
            queue before consuming them."""
            st = {"done": set(), "rate": 2 if first else 1}
            st["x"] = xpool.tile([P, cfg.KO, cfg.N], F16, tag="xT",
                                 name=f"x_{rep}_{b}")
            # x arrives in column halves, first half (consumed by the first
            # two projection groups) fully ahead of the second.
            half_n = cfg.N // 2
            for hx in range(2):
                for ko in range(cfg.KO):
                    nc.sync.dma_start(
                        out=st["x"][:, ko, hx * half_n:(hx + 1) * half_n],
                        in_=xT[b, ko * P:(ko + 1) * P,
                               hx * half_n:(hx + 1) * half_n],
                    )
            for name in ("q", "k", "v"):
                st[name] = qkv.tile([cfg.HD2, cfg.N], F16, tag=f"{name}T",
                                    name=f"{name}T_{rep}_{b}")
            st["vaug"] = qkv.tile([P, cfg.NJ, 2 * DA], F16, tag="vaug",
                                  name=f"vaug_{rep}_{b}")
            ones_col = consts.tile([P, cfg.NJ], F16,
                                   name=f"ones_{rep}_{b}", tag="ones")
            nc.vector.memset(ones_col, 1.0)
            nc.vector.tensor_copy(out=st["vaug"][:, :, D], in_=ones_col)
            nc.vector.tensor_copy(out=st["vaug"][:, :, DA + D], in_=ones_col)

            def proj_half(name, it, half):
                def f():
                    key = f"ps_{name}_{it}"
                    if half == 0:
                        st[key] = ppool.tile([P, IT], F32, tag="proj",
                                             name=key)
                    ps = st[key]
                    kos = range(cfg.KO // 2) if half == 0 else \
                        range(cfg.KO // 2, cfg.KO)
                    for ko in kos:
                        nc.tensor.matmul(
                            ps[:cfg.HD2],
                            lhsT=w_sb[name][:, ko],
                            rhs=st["x"][:, ko, it * IT:(it + 1) * IT],
                            start=(ko == 0),
                            stop=(ko == cfg.KO - 1),
                        )
                    if half == 1:
                        if name == "v":
                            # no bias on device (host adds bv post-softmax)
                            nc.vector.tensor_copy(
                                out=st[name][:, it * IT:(it + 1) * IT],
                                in_=ps[:cfg.HD2],
                            )
                        else:
                            nc.vector.tensor_scalar_add(
                                out=st[name][:, it * IT:(it + 1) * IT],
                                in0=ps[:cfg.HD2],
                                scalar1=b_sb[name],
                            )
                        del st[key]
                        st["done"].add(f"{name}{it}")
                return f

            def transpose_jc(jc):
                def f():
                    if cfg.DMA_T:
                        # XBAR transpose straight into vaug (no PE, no copies)
                        for h in range(2):
                            nc.sync.dma_start(
                                out=st["vaug"][:, jc, h * DA:h * DA + D],
                                in_=st["v"][h * D:(h + 1) * D,
                                            jc * P:(jc + 1) * P],
                                transpose=True,
                            )
                    else:
                        tp = ppool.tile([P, IT], F16, tag="proj",
                                        name=f"tp_{jc}")
                        nc.tensor.transpose(
                            tp[:, :P], st["v"][:, jc * P:(jc + 1) * P],
                            identity
                        )
                        nc.vector.tensor_copy(out=st["vaug"][:, jc, 0:D],
                                              in_=tp[:, 0:D])
                        nc.vector.tensor_copy(out=st["vaug"][:, jc, DA:DA + D],
                                              in_=tp[:, D:cfg.HD2])
                    st["done"].add(f"t{jc}")
                return f

            NG = cfg.N // IT        # proj groups per name
            IPP = IT // P           # transposes per v group
            if first:
                # q0/k0 run inline so the first QK/exp can start right after
                # the x halves land; everything else trails into attention
                # with group g's work ordered just ahead of its consumers.
                for name in ("q", "k"):
                    proj_half(name, 0, 0)()
                    proj_half(name, 0, 1)()
                for g in range(NG):
                    if g > 0:
                        work_q.append(proj_half("q", g, 0))
                        work_q.append(proj_half("q", g, 1))
                        work_q.append(proj_half("k", g, 0))
                        work_q.append(proj_half("k", g, 1))
                    work_q.append(proj_half("v", g, 0))
                    work_q.append(proj_half("v", g, 1))
                    for jc in range(g * IPP, (g + 1) * IPP):
                        work_q.append(transpose_jc(jc))
            else:
                for name in ("q", "k", "v"):
                    for it in range(NG):
                        work_q.append(proj_half(name, it, 0))
                        work_q.append(proj_half(name, it, 1))
                for jc in range(cfg.NJ):
                    work_q.append(transpose_jc(jc))
            return st

        def drain(n):
            for _ in range(min(n, len(work_q))):
                work_q.popleft()()

        def attention(rep, b, st):
            """Global-j software pipeline: QK/exp run LAG j-chunks ahead of
            PV, carried across i-tile boundaries so the per-tile PV flush
            never waits on a just-produced exp. The PV accumulator pair is
            allocated lazily at its first PV so at most two i-tiles' accs
            are live alongside the interleaved projection psum."""
            def ensure(key):
                # emission order is program order: pull the producer of
                # `key` through the queue before emitting its consumer
                while key not in st["done"] and work_q:
                    work_q.popleft()()
                assert key in st["done"], f"missing producer {key}"

            qTl, kTl, v_aug = st["q"], st["k"], st["vaug"]
            o_sb = [opool.tile([DA, cfg.N], F32, tag="o",
                               name=f"o_{rep}_{b}_{h}") for h in range(2)]
            NTOT = cfg.NI * cfg.NJ
            e_hist = deque()
            acc = {}

            def emit_qk_exp(gj):
                it, j = divmod(gj, cfg.NJ)
                isl = slice(it * IT, (it + 1) * IT)
                ensure(f"q{it}")
                ensure(f"k{(j * P) // IT}")
                jsl = slice(j * P, (j + 1) * P)
                s = spool.tile([P, 2 * IT], F32, tag="s")
                qk_w = IT // cfg.QK_SPLIT
                for h in range(2):
                    hsl = slice(h * D, (h + 1) * D)
                    for sp in range(cfg.QK_SPLIT):
                        nc.tensor.matmul(
                            s[:, h * IT + sp * qk_w:h * IT + (sp + 1) * qk_w],
                            lhsT=kTl[hsl, jsl],
                            rhs=qTl[hsl, it * IT + sp * qk_w:
                                    it * IT + (sp + 1) * qk_w],
                        )
                e = epool.tile([P, 2 * IT], F16, tag="e")
                if cfg.EXP_STUB:
                    nc.scalar.activation(
                        e[:, 0:cfg.EXP_STUB], s[:, 0:cfg.EXP_STUB],
                        mybir.ActivationFunctionType.Exp)
                elif cfg.DVE_EVERY and j % cfg.DVE_EVERY == cfg.DVE_EVERY - 1:
                    k4t = k4pool.tile([P, 2 * IT], F32, tag="k4")
                    _emit_dve_exp(nc, k4t, e, s, c_tile)
                else:
                    nc.scalar.activation(
                        e, s, mybir.ActivationFunctionType.Exp)
                e_hist.append((gj, e))

            def emit_pv(gj, e):
                it, j = divmod(gj, cfg.NJ)
                if j == 0:
                    atag = "acc" if cfg.SPLIT_POOLS else "proj"
                    acc[it] = [apool.tile([DA, IT], F32, tag=atag,
                                          name=f"acc_{it}_{h}")
                               for h in range(2)]
                ensure(f"t{j}")
                pv_w = IT // cfg.PV_SPLIT
                for h in range(2):
                    for sp in range(cfg.PV_SPLIT):
                        nc.tensor.matmul(
                            acc[it][h][:, sp * pv_w:(sp + 1) * pv_w],
                            lhsT=v_aug[:, j, h * DA:(h + 1) * DA],
                            rhs=e[:, h * IT + sp * pv_w:
                                  h * IT + (sp + 1) * pv_w],
                            start=(j == 0),
                            stop=(j == cfg.NJ - 1),
                        )
                if j == cfg.NJ - 1:
                    isl = slice(it * IT, (it + 1) * IT)
                    for h in range(2):
                        nc.vector.tensor_copy(out=o_sb[h][:, isl],
                                              in_=acc[it][h])
                    del acc[it]

            for gj in range(NTOT + cfg.LAG):
                if gj < NTOT:
                    emit_qk_exp(gj)
                if len(e_hist) > cfg.LAG or gj >= NTOT:
                    gjp, ep = e_hist.popleft()
                    emit_pv(gjp, ep)
                if gj < NTOT:
                    drain(st["rate"] if gj < cfg.NJ else 1)
            for h in range(2):
                nc.sync.dma_start(out=outT[b, h], in_=o_sb[h])

        jobs = [(rep, b) for rep in range(repeat) for b in range(cfg.B)]
        st_cur = start_job(*jobs[0], first=True)
        for idx, (rep, b) in enumerate(jobs):
            if idx > 0:
                # flush this job's remaining prep (normally already drained
                # during the previous attention) before enqueueing the next
                drain(len(work_q))
            st_next = start_job(*jobs[idx + 1]) if idx + 1 < len(jobs) else None
            attention(rep, b, st_cur)
            st_cur = st_next
        drain(len(work_q))


def build_program(cfg: Cfg, repeat: int = 1):
    _register_dve_exp_ops()
    nc = bacc.Bacc("TRN2", target_bir_lowering=False, debug=False)
    io = {
        "xT": nc.dram_tensor("xT", (cfg.B, cfg.C, cfg.N), F16, kind="ExternalInput").ap(),
        "wq": nc.dram_tensor("wq", (cfg.C, cfg.HD2), F16, kind="ExternalInput").ap(),
        "wk": nc.dram_tensor("wk", (cfg.C, cfg.HD2), F16, kind="ExternalInput").ap(),
        "wv": nc.dram_tensor("wv", (cfg.C, cfg.HD2), F16, kind="ExternalInput").ap(),
        "bq": nc.dram_tensor("bq", (cfg.HD2, 1), F32, kind="ExternalInput").ap(),
        "bk": nc.dram_tensor("bk", (cfg.HD2, 1), F32, kind="ExternalInput").ap(),
        "outT": nc.dram_tensor(
            "outT", (cfg.B, 2, cfg.D + 1, cfg.N), F32, kind="ExternalOutput"
        ).ap(),
    }
    with tile.TileContext(nc) as tc:
        build_attention(tc, io, cfg, repeat=repeat)
    nc.compile()
    return nc


def shard_inputs(x, W_qkv, b_qkv, n_cores=8):
    """Full inputs -> per-core in_maps (head-sharded, q pre-scaled, fp16)."""
    B, N, C = x.shape
    D = 64
    H = C // D
    heads_per_core = H // n_cores
    assert heads_per_core == 2
    scale = D ** -0.5
    xT = np.ascontiguousarray(
        np.transpose(x, (0, 2, 1)), dtype=np.float32).astype(np.float16)
    W = np.asarray(W_qkv, np.float32).reshape(C, 3, H, D)
    bias = np.asarray(b_qkv, np.float32).reshape(3, H, D)
    in_maps = []
    for c in range(n_cores):
        hs = slice(2 * c, 2 * c + 2)
        in_maps.append({
            "xT": xT,
            "wq": np.ascontiguousarray(
                W[:, 0, hs].reshape(C, 128) * scale).astype(np.float16),
            "wk": np.ascontiguousarray(
                W[:, 1, hs].reshape(C, 128)).astype(np.float16),
            "wv": np.ascontiguousarray(
                W[:, 2, hs].reshape(C, 128)).astype(np.float16),
            "bq": np.ascontiguousarray(bias[0, hs].reshape(128, 1) * scale),
            "bk": np.ascontiguousarray(bias[1, hs].reshape(128, 1)),
        })
    return in_maps


def gather_output(results, b_qkv=None, B=2, N=2048, C=1024):
    """Per-core outT [B, 2, 65, N] -> full [B, N, C] (normalize + v-bias +
    interleave)."""
    bv = np.asarray(b_qkv, np.float32).reshape(3, C // 64, 64)[2] \
        if b_qkv is not None else None
    outs = []
    for ci, res in enumerate(results):
        oT = np.asarray(res["outT"], np.float32)   # [B, 2, 65, N]
        o = oT[:, :, :64, :] / oT[:, :, 64:65, :]  # [B, 2, 64, N]
        if bv is not None:
            o = o + bv[2 * ci:2 * ci + 2][None, :, :, None]
        outs.append(np.transpose(o, (0, 3, 1, 2)))  # [B, N, 2, 64]
    out = np.concatenate(outs, axis=2)              # [B, N, 16, 64]
    return np.ascontiguousarray(out.reshape(B, N, C))


_PROGRAM = None


def kernel(x, W_qkv, b_qkv):
    global _PROGRAM
    cfg = Cfg()
    x = np.asarray(x, np.float32)
    in_maps = shard_inputs(x, W_qkv, b_qkv)
    if _PROGRAM is None:
        _PROGRAM = build_program(cfg)
    res = run_bass_kernel_spmd(_PROGRAM, in_maps, core_ids=list(range(8)))
    return gather_output(res.results, b_qkv, cfg.B, cfg.N, cfg.C)


if __name__ == "__main__":
    rng = np.random.default_rng(0)
    x = rng.standard_normal((2, 2048, 1024), dtype=np.float32)
    W = rng.standard_normal((1024, 3072), dtype=np.float32) * (1024 ** -0.5)
    b = rng.standard_normal(3072, dtype=np.float32) * 0.01
    out = kernel(x, W, b)
    print(out.shape, out.dtype, float(np.abs(out).max()))


# revision 15
# speedup vs baseline: 1.7349x; 1.7349x over previous
"""Multi-head attention (QKV projection + softmax attention) on 8 TRN2 NeuronCores.

Problem: x[2,2048,1024] @ W_qkv[1024,3072] + b -> q,k,v (16 heads, d=64);
out = softmax(q k^T / sqrt(d)) v, returned as [2,2048,1024] fp32.

Sharding: head-parallel. Core c owns heads {2c, 2c+1} for both batches.
Each core computes the QKV projection only for its own heads' columns of
W_qkv and runs its 4 (batch, head) attention instances locally. No
collectives; host gathers/normalizes/concatenates.

Device-side layout (per core):
  - x is fed pre-transposed AND pre-cast to fp16 (xT [B, C, N]) so the
    projection produces qT/kT [d, n] (head dim on partitions, h0 on 0-63,
    h1 on 64-127) directly with full-width matmuls.
  - All matmul operands are fp16 (PSUM accumulation stays fp32). fp16 runs
    at the same 1 col/cycle PE rate as f32r but gets FWL (fast weight
    load, 2 elems per 32-bit read) and halves SBUF/DMA traffic. Numerics
    sim: end-to-end max rel err ~1.2e-3 vs the fp32 reference (gate 2e-2).
  - The two heads' QK matmuls are row-tiled (h0 rows 0-63, h1 rows 64-127,
    via base_partition-inferred tile_position), so they run concurrently
    on disjoint PE row strips.
  - Attention computes S^T = K Q^T per head via matmul(lhsT=kT, rhs=qT),
    softmax numerator via exp per j-chunk covering both heads, and
    out^T = V^T E^T via matmul(lhsT=[v|1], rhs=E^T) accumulated over j in
    PSUM. The appended ones-column yields the softmax denominator as row
    64 of the accumulator; normalization happens on host.
  - The v bias is NOT applied on device: softmax weights sum to 1, so
    out = sum_k w_k (v_k + bv) = (sum_k w_k v_k) + bv, and the host adds
    bv after normalization.

Performance structure (found via on-HW A/B on the f32r baseline): engine
throughputs are fine, but the per-j QK->exp->PV cross-engine handoff
latency is very expensive on this hardware. Two mitigations:
  - PV consumes e with a lag of 3 j-chunks (deep software pipeline).
  - every DVE_EVERY-th j-chunk's exp runs on the Vector engine instead of
    ACT, via two custom DVE ops computing (L(s)*(K(s)*c)^4)^8 = K^32*L^8
    ~ exp(s) to 1e-5 rel (exactly scale-1 so ACT/DVE rows mix freely in
    one softmax column). This splits the s-buffer WAR chain across two
    independent engines, amortizing the handoff latency.
"""

import math
import os
from contextlib import ExitStack
from dataclasses import dataclass

import numpy as np

import concourse.bass as bass
import concourse.tile as tile
from concourse import bacc, mybir
from concourse.bass_utils import run_bass_kernel_spmd
from concourse.masks import make_identity

F32 = mybir.dt.float32
F16 = mybir.dt.float16

# ---------------------------------------------------------------------------
# Custom DVE exp: e = (L(s) * (K(s)*c)^4)^8 = K^32 * L^8 ~ exp(s), |s|<=8.5
#   K(s) = ((s+A)^2 + B) * (s+D)   op1 EXPK4_ANT: ((K*c))^4   (8 ALU ops)
#   L(s) = ((s+E)^2 + F) * G       op2 EXPFIN_ANT: (L*Src1)^8 (8 ALU ops)
# c = 2^-18 keeps intermediates O(1); G folds the fit constant so the
# overall scale is exactly 1. fp32 end-to-end max rel err vs exp: 9.6e-6.

EXP_A = 40.82381078942194
EXP_B = 2861.2185768771237
EXP_D = 63.39710432711229
EXP_C = 3.814697265625e-06      # 2^-18
EXP_E = -16.140267142823447
EXP_F = 2896.6915957224187
EXP_G = 0.00022030800137427044

_DVE_OPS = {}


def _register_dve_exp_ops():
    if _DVE_OPS:
        return
    import concourse.dve_ops as dve_ops_mod
    from concourse.dve_ops import OPS, DveOp
    from concourse.dve_spec import (Spec, Src0, Src1, C0, C1, C2, C3, sq,
                                    lower, _spill_c3_to_src1)
    from concourse.dve_uop import DveOpSpec

    by_name = {op.name: op for op in OPS}
    if "EXPK4_ANT" in by_name:
        _DVE_OPS["k4"] = by_name["EXPK4_ANT"]
        _DVE_OPS["fin"] = by_name["EXPFIN_ANT"]
        return

    k4_body = _spill_c3_to_src1(
        sq(sq((sq(Src0 + C0) + C1) * ((Src0 + C2) * C3))))
    K4 = DveOp(
        "EXPK4_ANT",
        Spec(
            body=k4_body,
            reference=lambda in0, in1, s0, s1, imm2: (
                (((in0 + s0) ** 2 + s1) * ((in0 + imm2) * in1)) ** 4),
        ),
        subdim=False,
        uops_sha={},
    )
    fin_body = sq(sq(sq(((sq(Src0 + C0) + C1) * C2) * Src1)))
    FIN = DveOp(
        "EXPFIN_ANT",
        Spec(
            body=fin_body,
            reference=lambda in0, in1, s0, s1, imm2: (
                ((((in0 + s0) ** 2 + s1) * imm2) * in1) ** 8),
        ),
        subdim=False,
        uops_sha={},
    )
    for op in (K4, FIN):
        for ver in ("v3", "v4"):
            spec = DveOpSpec(name=op.name, opcode=1,
                             uops=lower(op.spec, ver=ver), rd1_en=True)
            op.uops_sha[ver] = spec.sha(ver)
    OPS.append(K4)
    OPS.append(FIN)
    base = dve_ops_mod._CUSTOM_DVE_ROW_BASE
    for i, op in enumerate(OPS):
        dve_ops_mod._SUB_OPCODE_FOR_NAME[op.name] = base + i
    assert max(dve_ops_mod._SUB_OPCODE_FOR_NAME.values()) < 0x20
    _DVE_OPS["k4"] = K4
    _DVE_OPS["fin"] = FIN


def _emit_dve_exp(nc, k4_tile, out, s_ap, c_tile):
    """out (fp16 SBUF) = exp(s_ap); k4_tile fp32 scratch, c_tile [P,1]=c."""
    nc.vector._custom_dve(_DVE_OPS["k4"], out=k4_tile, in0=s_ap, in1=c_tile,
                          s0=EXP_A, s1=EXP_B, imm2=EXP_D)
    nc.vector._custom_dve(_DVE_OPS["fin"], out=out, in0=s_ap, in1=k4_tile,
                          s0=EXP_E, s1=EXP_F, imm2=EXP_G)


@dataclass(frozen=True)
class Cfg:
    B: int = 2          # batches
    N: int = 2048       # sequence length
    C: int = 1024       # model dim (contraction dim of the projection)
    D: int = 64         # head dim
    IT: int = 512       # i-tile (query block, moving-dim of S^T / PV matmuls)
    P: int = 128        # partitions
    LAG: int = 3        # PV consumes e this many j-chunks behind exp
    EBUFS: int = 5      # e-tile ring depth
    DVE_EVERY: int = 4  # every k-th j-chunk's exp runs on the DVE
    EXP_STUB: int = 0   # bench-only: exp writes just 1 column (timing A/B)
    QK_SPLIT: int = 1   # bench: split each QK matmul into this many pieces
    PV_SPLIT: int = 1   # bench: split each PV matmul into this many pieces
    DMA_T: int = 0      # transpose v via DMA XBAR instead of the PE
    SPLIT_POOLS: int = 1  # PV accumulators get their own PSUM ring (2 banks)
    XQ: int = 0         # x-load DMA queues: 0=sync only, 1=alternate sync/gpsimd
    CONST_W: int = 0    # bench-only: constant QK/PV stationaries (timing A/B)

    @property
    def KO(self):       # k-chunks in the projection contraction
        return self.C // self.P

    @property
    def NJ(self):       # key chunks of 128
        return self.N // self.P

    @property
    def NI(self):       # query tiles
        return self.N // self.IT

    @property
    def HD2(self):      # two heads stacked on partitions
        return 2 * self.D


def build_attention(tc: tile.TileContext, io: dict, cfg: Cfg, repeat: int = 1):
    nc = tc.nc
    P, D, IT = cfg.P, cfg.D, cfg.IT
    xT, wq, wk, wv, bq, bk, outT = (
        io["xT"], io["wq"], io["wk"], io["wv"], io["bq"], io["bk"], io["outT"],
    )

    with ExitStack() as ctx:
        consts = ctx.enter_context(tc.tile_pool(name="consts", bufs=1))
        xpool = ctx.enter_context(tc.tile_pool(name="xpool", bufs=1))
        qkv = ctx.enter_context(tc.tile_pool(name="qkv", bufs=2))
        epool = ctx.enter_context(tc.tile_pool(name="epool", bufs=cfg.EBUFS))
        opool = ctx.enter_context(tc.tile_pool(name="opool", bufs=4))
        k4pool = ctx.enter_context(tc.tile_pool(name="k4pool", bufs=2))
        # PSUM budget (8 banks): spool 2x [128, 2*IT] = 4 banks; the PV
        # accumulators get their own 2-buf ring (they are held for a whole
        # i-row, so sharing a ring with short-lived proj/transpose tiles
        # would inject long WAR stalls into the in-order PE queue); the
        # projection psum groups and transpose outputs share a 2-buf ring.
        if cfg.SPLIT_POOLS:
            ppool = ctx.enter_context(tc.tile_pool(name="ppool", bufs=2, space="PSUM"))
            spool = ctx.enter_context(tc.tile_pool(name="spool", bufs=2, space="PSUM"))
            apool = ctx.enter_context(tc.tile_pool(name="apool", bufs=2, space="PSUM"))
        else:
            ppool = ctx.enter_context(tc.tile_pool(name="ppool", bufs=4, space="PSUM"))
            spool = ctx.enter_context(tc.tile_pool(name="spool", bufs=2, space="PSUM"))
            apool = ppool

        identity = consts.tile([P, P], F16)
        make_identity(nc, identity)

        c_tile = consts.tile([P, 1], F32, name="c_exp")
        nc.vector.memset(c_tile, EXP_C)

        # weights [ki, ko, m] fp16 (host pre-cast) and biases [p, 1] fp32.
        w_sb = {}
        for name, wdram in (("q", wq), ("k", wk), ("v", wv)):
            w_sb[name] = consts.tile([P, cfg.KO, cfg.HD2], F16, name=f"w_{name}")
            nc.sync.dma_start(
                out=w_sb[name],
                in_=wdram.rearrange("(ko ki) m -> ki ko m", ki=P),
            )
        b_sb = {}
        for name, bdram in (("q", bq), ("k", bk)):
            b_sb[name] = consts.tile([cfg.HD2, 1], F32, name=f"b_{name}")
            nc.sync.dma_start(out=b_sb[name], in_=bdram)

        DA = D + 1  # head-dim columns + appended ones column

        from collections import deque

        work_q = deque()

        def start_job(rep, b, first=False):
            """Allocate batch tiles, emit the x-load DMAs, and enqueue the
            projection/transpose work thunks. The matmuls drain into the
            surrounding attention loops; emission order is program order, so
            attention uses ensure() to pull required producers through the
            queue before consuming them."""
            st = {"done": set(), "rate": 2 if first else 1}
            st["x"] = xpool.tile([P, cfg.KO, cfg.N], F16, tag="xT",
                                 name=f"x_{rep}_{b}")
            # x arrives in column halves, first half (consumed by the first
            # two projection groups) fully ahead of the second.
            half_n = cfg.N // 2
            for hx in range(2):
                for ko in range(cfg.KO):
                    eng = nc.gpsimd if (cfg.XQ and ko % 2) else nc.sync
                    eng.dma_start(
                        out=st["x"][:, ko, hx * half_n:(hx + 1) * half_n],
                        in_=xT[b, ko * P:(ko + 1) * P,
                               hx * half_n:(hx + 1) * half_n],
                    )
            for name in ("q", "k", "v"):
                st[name] = qkv.tile([cfg.HD2, cfg.N], F16, tag=f"{name}T",
                                    name=f"{name}T_{rep}_{b}")
            st["vaug"] = qkv.tile([P, cfg.NJ, 2 * DA], F16, tag="vaug",
                                  name=f"vaug_{rep}_{b}")
            ones_col = consts.tile([P, cfg.NJ], F16,
                                   name=f"ones_{rep}_{b}", tag="ones")
            nc.vector.memset(ones_col, 1.0)
            nc.vector.tensor_copy(out=st["vaug"][:, :, D], in_=ones_col)
            nc.vector.tensor_copy(out=st["vaug"][:, :, DA + D], in_=ones_col)

            def proj_half(name, it, half):
                def f():
                    key = f"ps_{name}_{it}"
                    if half == 0:
                        st[key] = ppool.tile([P, IT], F32, tag="proj",
                                             name=key)
                    ps = st[key]
                    kos = range(cfg.KO // 2) if half == 0 else \
                        range(cfg.KO // 2, cfg.KO)
                    for ko in kos:
                        nc.tensor.matmul(
                            ps[:cfg.HD2],
                            lhsT=w_sb[name][:, ko],
                            rhs=st["x"][:, ko, it * IT:(it + 1) * IT],
                            start=(ko == 0),
                            stop=(ko == cfg.KO - 1),
                        )
                    if half == 1:
                        if name == "v":
                            # no bias on device (host adds bv post-softmax)
                            nc.vector.tensor_copy(
                                out=st[name][:, it * IT:(it + 1) * IT],
                                in_=ps[:cfg.HD2],
                            )
                        else:
                            nc.vector.tensor_scalar_add(
                                out=st[name][:, it * IT:(it + 1) * IT],
                                in0=ps[:cfg.HD2],
                                scalar1=b_sb[name],
                            )
                        del st[key]
                        st["done"].add(f"{name}{it}")
                return f

            def transpose_jc(jc):
                def f():
                    if cfg.DMA_T:
                        # XBAR transpose straight into vaug (no PE, no copies)
                        for h in range(2):
                            nc.sync.dma_start(
                                out=st["vaug"][:, jc, h * DA:h * DA + D],
                                in_=st["v"][h * D:(h + 1) * D,
                                            jc * P:(jc + 1) * P],
                                transpose=True,
                            )
                    else:
                        tp = ppool.tile([P, IT], F16, tag="proj",
                                        name=f"tp_{jc}")
                        nc.tensor.transpose(
                            tp[:, :P], st["v"][:, jc * P:(jc + 1) * P],
                            identity
                        )
                        nc.vector.tensor_copy(out=st["vaug"][:, jc, 0:D],
                                              in_=tp[:, 0:D])
                        nc.vector.tensor_copy(out=st["vaug"][:, jc, DA:DA + D],
                                              in_=tp[:, D:cfg.HD2])
                    st["done"].add(f"t{jc}")
                return f

            NG = cfg.N // IT        # proj groups per name
            IPP = IT // P           # transposes per v group
            if first:
                # q0/k0 run inline so the first QK/exp can start right after
                # the x halves land; everything else trails into attention
                # with group g's work ordered just ahead of its consumers.
                for name in ("q", "k"):
                    proj_half(name, 0, 0)()
                    proj_half(name, 0, 1)()
                for g in range(NG):
                    if g > 0:
                        work_q.append(proj_half("q", g, 0))
                        work_q.append(proj_half("q", g, 1))
                        work_q.append(proj_half("k", g, 0))
                        work_q.append(proj_half("k", g, 1))
                    work_q.append(proj_half("v", g, 0))
                    work_q.append(proj_half("v", g, 1))
                    for jc in range(g * IPP, (g + 1) * IPP):
                        work_q.append(transpose_jc(jc))
            else:
                for name in ("q", "k", "v"):
                    for it in range(NG):
                        work_q.append(proj_half(name, it, 0))
                        work_q.append(proj_half(name, it, 1))
                for jc in range(cfg.NJ):
                    work_q.append(transpose_jc(jc))
            return st

        def drain(n):
            for _ in range(min(n, len(work_q))):
                work_q.popleft()()

        def attention(rep, b, st):
            """Global-j software pipeline: QK/exp run LAG j-chunks ahead of
            PV, carried across i-tile boundaries so the per-tile PV flush
            never waits on a just-produced exp. The PV accumulator pair is
            allocated lazily at its first PV so at most two i-tiles' accs
            are live alongside the interleaved projection psum."""
            def ensure(key):
                # emission order is program order: pull the producer of
                # `key` through the queue before emitting its consumer
                while key not in st["done"] and work_q:
                    work_q.popleft()()
                assert key in st["done"], f"missing producer {key}"

            qTl, kTl, v_aug = st["q"], st["k"], st["vaug"]
            o_sb = [opool.tile([DA, cfg.N], F32, tag="o",
                               name=f"o_{rep}_{b}_{h}") for h in range(2)]
            NTOT = cfg.NI * cfg.NJ
            e_hist = deque()
            acc = {}

            def emit_qk_exp(gj):
                it, j = divmod(gj, cfg.NJ)
                isl = slice(it * IT, (it + 1) * IT)
                ensure(f"q{it}")
                ensure(f"k{(j * P) // IT}")
                jsl = slice(j * P, (j + 1) * P)
                s = spool.tile([P, 2 * IT], F32, tag="s")
                qk_w = IT // cfg.QK_SPLIT
                for h in range(2):
                    hsl = slice(h * D, (h + 1) * D)
                    ksl = slice(0, P) if cfg.CONST_W else jsl
                    for sp in range(cfg.QK_SPLIT):
                        nc.tensor.matmul(
                            s[:, h * IT + sp * qk_w:h * IT + (sp + 1) * qk_w],
                            lhsT=kTl[hsl, ksl],
                            rhs=qTl[hsl, it * IT + sp * qk_w:
                                    it * IT + (sp + 1) * qk_w],
                        )
                e = epool.tile([P, 2 * IT], F16, tag="e")
                if cfg.EXP_STUB:
                    nc.scalar.activation(
                        e[:, 0:cfg.EXP_STUB], s[:, 0:cfg.EXP_STUB],
                        mybir.ActivationFunctionType.Exp)
                elif cfg.DVE_EVERY and j % cfg.DVE_EVERY == cfg.DVE_EVERY - 1:
                    k4t = k4pool.tile([P, 2 * IT], F32, tag="k4")
                    _emit_dve_exp(nc, k4t, e, s, c_tile)
                else:
                    nc.scalar.activation(
                        e, s, mybir.ActivationFunctionType.Exp)
                e_hist.append((gj, e))

            def emit_pv(gj, e):
                it, j = divmod(gj, cfg.NJ)
                if j == 0:
                    atag = "acc" if cfg.SPLIT_POOLS else "proj"
                    acc[it] = [apool.tile([DA, IT], F32, tag=atag,
                                          name=f"acc_{it}_{h}")
                               for h in range(2)]
                ensure(f"t{j}")
                pv_w = IT // cfg.PV_SPLIT
                jv = 0 if cfg.CONST_W else j
                for h in range(2):
                    for sp in range(cfg.PV_SPLIT):
                        nc.tensor.matmul(
                            acc[it][h][:, sp * pv_w:(sp + 1) * pv_w],
                            lhsT=v_aug[:, jv, h * DA:(h + 1) * DA],
                            rhs=e[:, h * IT + sp * pv_w:
                                  h * IT + (sp + 1) * pv_w],
                            start=(j == 0),
                            stop=(j == cfg.NJ - 1),
                        )
                if j == cfg.NJ - 1:
                    isl = slice(it * IT, (it + 1) * IT)
                    for h in range(2):
                        nc.vector.tensor_copy(out=o_sb[h][:, isl],
                                              in_=acc[it][h])
                    del acc[it]

            for gj in range(NTOT + cfg.LAG):
                if gj < NTOT:
                    emit_qk_exp(gj)
                if len(e_hist) > cfg.LAG or gj >= NTOT:
                    gjp, ep = e_hist.popleft()
                    emit_pv(gjp, ep)
                if gj < NTOT:
                    drain(st["rate"] if gj < cfg.NJ else 1)
            for h in range(2):
                nc.sync.dma_start(out=outT[b, h], in_=o_sb[h])

        jobs = [(rep, b) for rep in range(repeat) for b in range(cfg.B)]
        st_cur = start_job(*jobs[0], first=True)
        for idx, (rep, b) in enumerate(jobs):
            if idx > 0:
                # flush this job's remaining prep (normally already drained
                # during the previous attention) before enqueueing the next
                drain(len(work_q))
            st_next = start_job(*jobs[idx + 1]) if idx + 1 < len(jobs) else None
            attention(rep, b, st_cur)
            st_cur = st_next
        drain(len(work_q))


def build_program(cfg: Cfg, repeat: int = 1):
    _register_dve_exp_ops()
    nc = bacc.Bacc("TRN2", target_bir_lowering=False, debug=False)
    io = {
        "xT": nc.dram_tensor("xT", (cfg.B, cfg.C, cfg.N), F16, kind="ExternalInput").ap(),
        "wq": nc.dram_tensor("wq", (cfg.C, cfg.HD2), F16, kind="ExternalInput").ap(),
        "wk": nc.dram_tensor("wk", (cfg.C, cfg.HD2), F16, kind="ExternalInput").ap(),
        "wv": nc.dram_tensor("wv", (cfg.C, cfg.HD2), F16, kind="ExternalInput").ap(),
        "bq": nc.dram_tensor("bq", (cfg.HD2, 1), F32, kind="ExternalInput").ap(),
        "bk": nc.dram_tensor("bk", (cfg.HD2, 1), F32, kind="ExternalInput").ap(),
        "outT": nc.dram_tensor(
            "outT", (cfg.B, 2, cfg.D + 1, cfg.N), F32, kind="ExternalOutput"
        ).ap(),
    }
    with tile.TileContext(nc) as tc:
        build_attention(tc, io, cfg, repeat=repeat)
    nc.compile()
    return nc


def shard_inputs(x, W_qkv, b_qkv, n_cores=8):
    """Full inputs -> per-core in_maps (head-sharded, q pre-scaled, fp16)."""
    B, N, C = x.shape
    D = 64
    H = C // D
    heads_per_core = H // n_cores
    assert heads_per_core == 2
    scale = D ** -0.5
    xT = np.ascontiguousarray(
        np.transpose(x, (0, 2, 1)), dtype=np.float32).astype(np.float16)
    W = np.asarray(W_qkv, np.float32).reshape(C, 3, H, D)
    bias = np.asarray(b_qkv, np.float32).reshape(3, H, D)
    in_maps = []
    for c in range(n_cores):
        hs = slice(2 * c, 2 * c + 2)
        in_maps.append({
            "xT": xT,
            "wq": np.ascontiguousarray(
                W[:, 0, hs].reshape(C, 128) * scale).astype(np.float16),
            "wk": np.ascontiguousarray(
                W[:, 1, hs].reshape(C, 128)).astype(np.float16),
            "wv": np.ascontiguousarray(
                W[:, 2, hs].reshape(C, 128)).astype(np.float16),
            "bq": np.ascontiguousarray(bias[0, hs].reshape(128, 1) * scale),
            "bk": np.ascontiguousarray(bias[1, hs].reshape(128, 1)),
        })
    return in_maps


def gather_output(results, b_qkv=None, B=2, N=2048, C=1024):
    """Per-core outT [B, 2, 65, N] -> full [B, N, C] (normalize + v-bias +
    interleave)."""
    bv = np.asarray(b_qkv, np.float32).reshape(3, C // 64, 64)[2] \
        if b_qkv is not None else None
    outs = []
    for ci, res in enumerate(results):
        oT = np.asarray(res["outT"], np.float32)   # [B, 2, 65, N]
        o = oT[:, :, :64, :] / oT[:, :, 64:65, :]  # [B, 2, 64, N]
        if bv is not None:
            o = o + bv[2 * ci:2 * ci + 2][None, :, :, None]
        outs.append(np.transpose(o, (0, 3, 1, 2)))  # [B, N, 2, 64]
    out = np.concatenate(outs, axis=2)              # [B, N, 16, 64]
    return np.ascontiguousarray(out.reshape(B, N, C))


_PROGRAM = None


def kernel(x, W_qkv, b_qkv):
    global _PROGRAM
    cfg = Cfg()
    x = np.asarray(x, np.float32)
    in_maps = shard_inputs(x, W_qkv, b_qkv)
    if _PROGRAM is None:
        _PROGRAM = build_program(cfg)
    res = run_bass_kernel_spmd(_PROGRAM, in_maps, core_ids=list(range(8)))
    return gather_output(res.results, b_qkv, cfg.B, cfg.N, cfg.C)


if __name__ == "__main__":
    rng = np.random.default_rng(0)
    x = rng.standard_normal((2, 2048, 1024), dtype=np.float32)
    W = rng.standard_normal((1024, 3072), dtype=np.float32) * (1024 ** -0.5)
    b = rng.standard_normal(3072, dtype=np.float32) * 0.01
    out = kernel(x, W, b)
    print(out.shape, out.dtype, float(np.abs(out).max()))
